# revision 8
# baseline (speedup 1.0000x reference)
"""AdaptiveMultiWIRE on 8 TRN2 NeuronCores — v2 (custom-DVE rebalance).

Changes vs v1 (point-major baseline, 576us):
  - g4 block dropped: hidden wave = 4H=724 cols [g4'|g1|g2|g3] with
    g4' = S*la.re (the g0 block pre-scaled by S in the WEIGHTS), so one
    unscaled Square covers all four magnitude terms.  L0 wave = 2H=362.
    PE hidden streaming drops 905->724 (-20%), L0 543->362.
  - custom DVE op FRAC_SCALE: fw = frac(psum*.c1) in ONE pass (replaces
    the k1 round + fw subtract pair): DVE psum drain halves.
  - custom DVE op SQ_SCALE_ADD: u = (in0*c0)^2 + in1*c1 straight from
    psum; 4-pass chains build wv per tile with no ScalarE/Pool help.
    Per-tile path assignment (A=ScalarE square + Pool v-adds,
    B=DVE chain, C=DVE evict + Pool square + v-adds) balances the three
    elementwise engines.
  - exps of both channels issue back-to-back per layer: 2 act-table
    loads per layer instead of 4.
  - xre via c2 = sh^2 (Pool), d = c2-1/2 (DVE ts at 4x SIMD), then
    Pool multiplies into the xq staging layout; ones region of xq is
    written once (xq no longer aliases the square staging buffer).
"""

import numpy as np

C, N, H, OUT, NIN, NSRC, NB = 16, 8192, 181, 3, 2, 32, 8
OMEGA, SCALE = 30.0, 10.0
NCORES, CPC = 8, 2
PI = float(np.pi)
R2 = OMEGA / (2.0 * PI)
S0 = SCALE / R2
EBIAS = OMEGA * OMEGA / (4.0 * SCALE * SCALE)
MAGIC = 12582912.0
PB = 2048
NBATCH = N // PB
PC = 128
NPC = PB // PC
NW = 4 * H                       # 724 hidden wave cols [g4'|g1|g2|g3]
NW0 = 2 * H                      # 362 L0 wave cols [S*la | S*lb]
KC = [(0, 128), (128, 256), (256, 363)]

# per-tile square-path assignment (8 psum tiles per layer-channel):
# 'A' = ScalarE square + Pool v-adds, 'B' = DVE sq_scale_add chain,
# 'C' = DVE evict + Pool square + Pool v-adds
PATH_H = ['B', 'B', 'D', 'A', 'D', 'B', 'A', 'A']
PATH_0 = ['B', 'B', 'B', 'B', 'B', 'B', 'A', 'A']

_GRAPH = None
_OPS = {}


def _register_ops():
    """Register the two custom DVE ops (idempotent)."""
    if _OPS:
        return _OPS
    from concourse.dve_spec import Spec, Src0, Src1, C0, C1, lower, sq
    from concourse.dve_uop import DveOpSpec
    from concourse import dve_ops as D

    def reg(name, spec):
        for op in D.OPS:
            if op.name == name:
                return op
        shas = {}
        for ver in ("v3", "v4"):
            try:
                uops = lower(spec, ver=ver)
                shas[ver] = DveOpSpec(name=name, opcode=31, uops=uops,
                                      rd1_en=True).sha(ver)
            except Exception:
                pass
        op = D.DveOp(name, spec, subdim=False, uops_sha=shas)
        D.OPS.append(op)
        D.CUSTOM_DVE_SPECS[name] = spec
        D._SUB_OPCODE_FOR_NAME[name] = D._CUSTOM_DVE_ROW_BASE + len(D.OPS) - 1
        return op

    def _ref_frac(in0, in1, c0, c1, c2):
        x = in0.astype(np.float32) * np.float32(c1)
        return x - ((x + np.float32(c0)) - np.float32(c0))

    _OPS["frac"] = reg("FRAC_SCALE_ANT", Spec(
        body=(Src0 * C1) - (((Src0 * C1) + C0) - C0),
        reference=_ref_frac,
    ))

    def _ref_sqsa(in0, in1, c0, c1, c2):
        x = in0.astype(np.float32) * np.float32(c0)
        y = in1.astype(np.float32)
        return x * x + y * np.float32(c1)

    _OPS["sqsa"] = reg("SQ_SCALE_ADD_ANT", Spec(
        body=sq(Src0 * C0) + Src1 * C1,
        reference=_ref_sqsa,
    ))
    return _OPS


def _build_graph():
    import concourse.mybir as mybir
    from concourse import bacc
    from concourse.tile import TileContext

    ops = _register_ops()
    FRAC, SQSA = ops["frac"], ops["sqsa"]

    dt = mybir.dt
    f16, f32 = dt.float16, dt.float32
    Alu = mybir.AluOpType
    Act = mybir.ActivationFunctionType

    nc = bacc.Bacc()
    xa_d = nc.declare_dram_parameter("xa", [CPC, 3, N], f16, isOutput=False)
    w0_d = nc.declare_dram_parameter("w0", [CPC, 3, NW0], f16, isOutput=False)
    w1_d = nc.declare_dram_parameter("w1", [CPC, 2 * H + 1, NW], f16,
                                     isOutput=False)
    w2_d = nc.declare_dram_parameter("w2", [CPC, 2 * H + 1, NW], f16,
                                     isOutput=False)
    wf_d = nc.declare_dram_parameter("wf", [CPC, 2 * H + 1, OUT], f16,
                                     isOutput=False)
    out_d = nc.declare_dram_parameter("out", [CPC, OUT, N], f16, isOutput=True)

    W = NPC * H          # 2896
    HW2 = W // 2

    with TileContext(nc) as tc:
        with (
            tc.tile_pool(name="wpool", bufs=1) as wpool,
            tc.tile_pool(name="xpool", bufs=1) as xpool,
            tc.tile_pool(name="apool", bufs=2) as apool,
            tc.tile_pool(name="spool", bufs=1) as spool,
            tc.tile_pool(name="psum", bufs=2, space="PSUM") as pp,
        ):
            # ---- persistent loads: w0 first (gates the first early);
            # the big hidden-layer weights stream afterwards -----------
            w0t, wts, wfts = [], [], []
            for ch in range(CPC):
                t = wpool.tile([3, NW0], f16, tag=f"w0{ch}", name=f"w0{ch}")
                nc.sync.dma_start(out=t[:], in_=w0_d[ch])
                w0t.append(t)
            def load_big_weights():
                for ch in range(CPC):
                    per_layer = []
                    for li, wd in ((1, w1_d), (2, w2_d)):
                        tiles = []
                        for ki, (r0, r1) in enumerate(KC):
                            t = wpool.tile([r1 - r0, NW], f16,
                                           tag=f"w{li}{ch}k{ki}")
                            nc.sync.dma_start(out=t[:], in_=wd[ch, r0:r1, :])
                            tiles.append(t)
                        per_layer.append(tiles)
                    wts.append(per_layer)
                    tiles = []
                    for ki, (r0, r1) in enumerate(KC):
                        t = wpool.tile([r1 - r0, OUT], f16, tag=f"wf{ch}k{ki}")
                        nc.sync.dma_start(out=t[:], in_=wf_d[ch, r0:r1, :])
                        tiles.append(t)
                    wfts.append(tiles)

            # zero tile: in1 operand for the c1=0 sq_scale_add passes
            zt = wpool.tile([128, 1448], f16, tag="zt", name="zt")
            nc.vector.memset(zt[:], 0.0)
            # persistent xq staging (ones region written once below)
            xqs = []
            for ch in range(CPC):
                xq = wpool.tile([128, 3 * PB], f16, tag=f"xq{ch}",
                                name=f"xq{ch}")
                q2 = xq[:, 2 * PB:3 * PB].rearrange("p (n r) -> p n r", r=128)
                nc.vector.memset(q2[:, :, 106:128], 1.0)
                xqs.append(xq)

            def expph(ch, wv):
                E = spool.tile([128, W], f16, tag=f"E{ch}")
                nc.scalar.activation(E[:], wv[:], Act.Exp, bias=0.0,
                                     scale=-1.0)
                return E

            def mm_early(ch, li, xan, X):
                """Matmuls + psum drain (frac + per-path squares/wv) +
                inline half-batch Sins.  Returns (s, sh, wv)."""
                nsec = 2 if li == 0 else 4
                gw = nsec * H                      # full wave per slot
                nw = NW0 if li == 0 else NW
                paths = PATH_0 if li == 0 else PATH_H
                fw = spool.tile([128, W], f16, tag=f"fw{ch}")
                s = spool.tile([128, W], f16, tag=f"s{ch}")
                sh = spool.tile([128, W], f16, tag=f"sh{ch}")
                wvw = spool.tile([128, W], f16, tag=f"wv{ch}")
                sqw = spool.tile([128, 4 * 1448], f16, tag=f"sqw{ch}")
                uw = spool.tile([128, 4 * 362], f16, tag=f"u{ch}")
                vsc = spool.tile([128, 1448], f16, tag=f"vs{ch}")
                ev = xpool.tile([128, 1448], f16, tag=f"ev{ch}",
                                name=f"ev{ch}")
                for t in range(NPC // 2):
                    ps = pp.tile([128, 2048], f32, tag="wav")
                    for slot in (0, 1):
                        pc = 2 * t + slot
                        o = slot * 1024
                        if li == 0:
                            lhs = xan[:, pc * PC:(pc + 1) * PC]
                            nc.tensor.matmul(ps[:, o:o + NW0], lhsT=lhs,
                                             rhs=w0t[ch][:, 0:NW0],
                                             start=True, stop=True)
                        else:
                            wk = wts[ch][li - 1]
                            T0, T1, T2 = X
                            for ki, xt in enumerate((T0, T1, T2)):
                                kr = KC[ki][1] - KC[ki][0]
                                lhs = xt[0:kr, pc * PC:(pc + 1) * PC]
                                nc.tensor.matmul(ps[:, o:o + 512],
                                                 lhsT=lhs, rhs=wk[ki][:, 0:512],
                                                 start=(ki == 0), stop=(ki == 2))
                                nc.tensor.matmul(ps[:, o + 512:o + NW],
                                                 lhsT=lhs, rhs=wk[ki][:, 512:NW],
                                                 start=(ki == 0), stop=(ki == 2))
                    ps3 = ps[:].rearrange("p (s w) -> p s w", w=1024)
                    tsl = slice(t * 2 * H, (t + 1) * 2 * H)
                    # phase: fw = frac(g4'/S0) in one custom DVE pass
                    nc.vector._custom_dve(FRAC, out=fw[:, tsl],
                                          in0=ps3[:, :, 0:H],
                                          s0=MAGIC, s1=1.0 / S0)
                    path = paths[t]
                    if path == 'B':
                        # fused square+add pairs straight from psum:
                        # pass1: u = [g4'^2 | g1^2]; pass2: v = u + [g2^2|g3^2]
                        # pass3 (Pool): wv = v[0:H] + v[H:2H]
                        rA = uw[:, (t % 2) * 724:(t % 2) * 724 + 724]
                        rB = vsc[:, (t % 2) * 724:(t % 2) * 724 + 724]
                        if li == 0:
                            nc.vector._custom_dve(SQSA, out=rA,
                                                  in0=ps3[:, :, 0:2 * H],
                                                  in1=zt[:, 0:724], s0=1.0,
                                                  s1=0.0)
                            nc.gpsimd.tensor_tensor(
                                wvw[:, tsl].rearrange("p (s g) -> p s g", g=H),
                                rA.rearrange("p (s g) -> p s g", g=2 * H)[:, :, 0:H],
                                rA.rearrange("p (s g) -> p s g", g=2 * H)[:, :, H:2 * H],
                                Alu.add)
                        else:
                            nc.vector._custom_dve(SQSA, out=rA,
                                                  in0=ps3[:, :, 0:2 * H],
                                                  in1=zt[:, 0:724], s0=1.0,
                                                  s1=0.0)
                            nc.vector._custom_dve(SQSA, out=rB,
                                                  in0=ps3[:, :, 2 * H:4 * H],
                                                  in1=rA, s0=1.0, s1=1.0)
                            nc.gpsimd.tensor_tensor(
                                wvw[:, tsl].rearrange("p (s g) -> p s g", g=H),
                                rB.rearrange("p (s g) -> p s g", g=2 * H)[:, :, 0:H],
                                rB.rearrange("p (s g) -> p s g", g=2 * H)[:, :, H:2 * H],
                                Alu.add)
                    else:
                        sqa = sqw[:, (t % 4) * 1448:(t % 4) * 1448 + 2 * gw]
                        if path == 'A':
                            nc.scalar.activation(sqa, ps3[:, :, 0:gw],
                                                 Act.Square, bias=0.0,
                                                 scale=1.0)
                        else:  # D: DVE square-evict (SQSA with c1=0)
                            nc.vector._custom_dve(SQSA, out=sqa,
                                                  in0=ps3[:, :, 0:gw],
                                                  in1=zt[:, 0:2 * gw],
                                                  s0=1.0, s1=0.0)
                        sq3 = sqa.rearrange("p (s g) -> p s g", g=gw)
                        if li == 0:
                            nc.gpsimd.tensor_tensor(
                                wvw[:, tsl].rearrange("p (s g) -> p s g", g=H),
                                sq3[:, :, 0:H], sq3[:, :, H:2 * H], Alu.add)
                        else:
                            v1 = vsc[:, (t % 2) * 724:(t % 2) * 724 + 362]
                            v2 = vsc[:, (t % 2) * 724 + 362:(t % 2) * 724 + 724]
                            nc.gpsimd.tensor_tensor(
                                v1.rearrange("p (s g) -> p s g", g=H),
                                sq3[:, :, 0:H], sq3[:, :, H:2 * H], Alu.add)
                            nc.gpsimd.tensor_tensor(
                                v2.rearrange("p (s g) -> p s g", g=H),
                                sq3[:, :, 2 * H:3 * H], sq3[:, :, 3 * H:4 * H],
                                Alu.add)
                            nc.gpsimd.tensor_tensor(wvw[:, tsl], v1, v2,
                                                    Alu.add)
                    if t == NPC // 4 - 1:
                        nc.scalar.activation(s[:, 0:HW2], fw[:, 0:HW2],
                                             Act.Sin, bias=0.0, scale=2 * PI)
                        nc.scalar.activation(sh[:, 0:HW2], fw[:, 0:HW2],
                                             Act.Sin, bias=0.0, scale=PI)
                nc.scalar.activation(s[:, HW2:W], fw[:, HW2:W], Act.Sin,
                                     bias=0.0, scale=2 * PI)
                nc.scalar.activation(sh[:, HW2:W], fw[:, HW2:W], Act.Sin,
                                     bias=0.0, scale=PI)
                return s, sh, wvw

            def combine_transpose(ch, s, sh, E):
                """Quarter-major: each quarter's xre/xim ops + 3 DMA
                transposes fire as soon as that quarter's inputs are
                ready, so the next early's first matmuls start earlier
                than with layer-wide combine ops."""
                xq = xqs[ch]
                c2 = spool.tile([128, W], f16, tag=f"c2{ch}")
                T0 = xpool.tile([128, PB], f16, tag=f"T0{ch}", name=f"T0{ch}")
                T1 = xpool.tile([128, PB], f16, tag=f"T1{ch}", name=f"T1{ch}")
                T2 = xpool.tile([128, PB], f16, tag=f"T2{ch}", name=f"T2{ch}")
                for part in range(4):
                    wr = slice(part * 724, (part + 1) * 724)
                    nc.gpsimd.tensor_tensor(c2[:, wr], sh[:, wr], sh[:, wr],
                                            Alu.mult)
                    nc.vector.tensor_scalar(sh[:, wr], c2[:, wr], 1.0, 0.5,
                                            Alu.mult, Alu.subtract)
                    psl = slice(part * 512, (part + 1) * 512)
                    q0 = xq[:, 0:PB][:, psl].rearrange(
                        "p (n r) -> p n r", r=128)
                    q1 = xq[:, PB:2 * PB][:, psl].rearrange(
                        "p (n r) -> p n r", r=128)
                    q2 = xq[:, 2 * PB:3 * PB][:, psl].rearrange(
                        "p (n r) -> p n r", r=128)
                    d3 = sh[:, wr].rearrange("p (n g) -> p n g", g=H)
                    E3 = E[:, wr].rearrange("p (n g) -> p n g", g=H)
                    s3 = s[:, wr].rearrange("p (n g) -> p n g", g=H)
                    nc.gpsimd.tensor_tensor(q0[:, :, :], d3[:, :, 0:128],
                                            E3[:, :, 0:128], Alu.mult)
                    nc.gpsimd.tensor_tensor(q1[:, :, 0:53], d3[:, :, 128:H],
                                            E3[:, :, 128:H], Alu.mult)
                    nc.gpsimd.tensor_tensor(q1[:, :, 53:128], E3[:, :, 0:75],
                                            s3[:, :, 0:75], Alu.mult)
                    nc.gpsimd.tensor_tensor(q2[:, :, 0:106], E3[:, :, 75:H],
                                            s3[:, :, 75:H], Alu.mult)
                    for sec, Tt in enumerate((T0, T1, T2)):
                        t3 = Tt[:, psl].rearrange("f (b r) -> f b r", r=128)
                        nc.sync.dma_start_transpose(
                            t3, xq[:, sec * PB + part * 512:
                                   sec * PB + (part + 1) * 512])
                return T0, T1, T2

            # ---- main loop -------------------------------------------
            def load_xa(nb):
                d = {}
                for ch in range(CPC):
                    t = apool.tile([3, PB], f16, tag=f"xa{ch}")
                    nc.sync.dma_start(out=t[:],
                                      in_=xa_d[ch, :, nb * PB:(nb + 1) * PB])
                    d[ch] = t
                return d

            X = {ch: None for ch in range(CPC)}

            def final(ch, nb):
                T0, T1, T2 = X[ch]
                psf = pp.tile([OUT, PB], f32, tag="wav")
                ob = spool.tile([OUT, PB], f16, tag=f"ob{ch}")
                for half in range(2):
                    for ni in (2 * half, 2 * half + 1):
                        sl = slice(ni * 512, (ni + 1) * 512)
                        for ki, xt in enumerate((T0, T1, T2)):
                            kr = KC[ki][1] - KC[ki][0]
                            nc.tensor.matmul(psf[:, sl], lhsT=wfts[ch][ki][:],
                                             rhs=xt[0:kr, sl],
                                             start=(ki == 0), stop=(ki == 2))
                    hs = slice(half * 1024, (half + 1) * 1024)
                    nc.vector.tensor_copy(ob[:, hs], psf[:, hs])
                    nc.sync.dma_start(
                        out=out_d[ch, :, nb * PB + half * 1024:
                                  nb * PB + (half + 1) * 1024],
                        in_=ob[:, hs])

            xan = load_xa(0)
            for nb in range(NBATCH):
                if nb == 0:
                    mid = {}
                    mid[0] = mm_early(0, 0, xan[0], X[0])
                    load_big_weights()
                    mid[1] = mm_early(1, 0, xan[1], X[1])
                xan_next = load_xa(nb + 1) if nb + 1 < NBATCH else None
                for li in (0, 1, 2):
                    # both channels' exps back-to-back: 2 table loads/layer
                    E0 = expph(0, mid[0][2])
                    E1 = expph(1, mid[1][2])
                    Es = (E0, E1)
                    for ch in range(CPC):
                        s_, sh_, _ = mid[ch]
                        X[ch] = combine_transpose(ch, s_, sh_, Es[ch])
                        if li < 2:
                            mid[ch] = mm_early(ch, li + 1, xan[ch], X[ch])
                        else:
                            final(ch, nb)
                            if xan_next is not None:
                                mid[ch] = mm_early(ch, 0, xan_next[ch],
                                                   X[ch])
                xan = xan_next
    nc.finalize()
    return nc


def _get_graph():
    global _GRAPH
    if _GRAPH is None:
        _GRAPH = _build_graph()
    return _GRAPH


def _pack_inputs(inp, indices, model_idx, bias_idx, W0a, b0a, W0b, b0b,
                 W1a, b1a, W1b, b1b, W2a, b2a, W2b, b2b, Wf, bf):
    """Host-side gather + weight packing (v2 wave layout).

    Hidden wave cols [g4'|g1|g2|g3]: g4' = S*la.re (phase g0 recovered
    as g4'/S0 inside the frac op), g1 = S*la.im + O/2S, g2 = S*lb.re,
    g3 = S*lb.im.  Rows are [xre'(181); xim'(181); ones].
    """
    cplx = lambda a: a[..., 0] + 1j * a[..., 1]

    def pack_hidden(Wa, Wb, ba, bb, alpha, beta):
        re_rows = np.concatenate([
            SCALE * alpha * Wa.real, SCALE * alpha * Wa.imag,
            SCALE * alpha * Wb.real, SCALE * alpha * Wb.imag], axis=1)
        im_rows = np.concatenate([
            -SCALE * beta * Wa.imag, SCALE * beta * Wa.real,
            -SCALE * beta * Wb.imag, SCALE * beta * Wb.real], axis=1)
        ones_row = np.concatenate([
            SCALE * ba.real, SCALE * ba.imag + OMEGA / (2 * SCALE),
            SCALE * bb.real, SCALE * bb.imag])[None, :]
        return np.concatenate([re_rows, im_rows, ones_row],
                              axis=0).astype(np.float16)

    a0, b0c = -2.0, 1.0
    a1, b1c = -2.0 * np.exp(EBIAS), np.exp(EBIAS)

    in_maps = []
    for core in range(NCORES):
        m = {k: [] for k in ("xa", "w0", "w1", "w2", "wf")}
        for j in range(CPC):
            c = core * CPC + j
            mi, bi = int(model_idx[c]), int(bias_idx[c])
            x = inp[int(indices[c])]
            m["xa"].append(np.concatenate(
                [x.T, np.ones((1, N), np.float32)], 0).astype(np.float16))
            w0blk = np.concatenate([
                np.concatenate([SCALE * W0a[mi], SCALE * W0b[mi]], axis=1),
                np.concatenate([SCALE * b0a[bi], SCALE * b0b[bi]])[None, :],
            ], axis=0)
            m["w0"].append(w0blk.astype(np.float16))
            m["w1"].append(pack_hidden(cplx(W1a[mi]), cplx(W1b[mi]),
                                       cplx(b1a[bi]), cplx(b1b[bi]), a0, b0c))
            m["w2"].append(pack_hidden(cplx(W2a[mi]), cplx(W2b[mi]),
                                       cplx(b2a[bi]), cplx(b2b[bi]), a1, b1c))
            Wfc, bfc = cplx(Wf[mi]), cplx(bf[bi])
            wfblk = np.concatenate([
                a1 * Wfc.real, -b1c * Wfc.imag, bfc.real[None, :]],
                axis=0).astype(np.float16)
            m["wf"].append(wfblk)
        in_maps.append({k: np.stack(v) for k, v in m.items()})
    return in_maps


def kernel(**inputs):
    inp = np.asarray(inputs["inp"], np.float32)
    args = {k: np.asarray(v) for k, v in inputs.items()}
    in_maps = _pack_inputs(
        inp, args["indices"], args["model_idx"], args["bias_idx"],
        *[np.asarray(args[k], np.float32) for k in
          ("W0a", "b0a", "W0b", "b0b", "W1a", "b1a", "W1b", "b1b",
           "W2a", "b2a", "W2b", "b2b", "Wf", "bf")])
    from concourse.bass_utils import run_bass_kernel_spmd
    nc = _get_graph()
    res = run_bass_kernel_spmd(nc, in_maps, core_ids=list(range(NCORES)))
    out = np.empty((1, C, N, OUT), np.float32)
    for core in range(NCORES):
        o = np.asarray(res.results[core]["out"])
        for j in range(CPC):
            out[0, core * CPC + j] = o[j].T.astype(np.float32)
    return out


if __name__ == "__main__":
    import jax
    import reference
    cpu = jax.devices("cpu")[0]
    with jax.default_device(cpu):
        ins = {k: np.asarray(v) for k, v in reference.setup_inputs().items()}
        exp = np.asarray(reference.reference(
            **{k: jax.device_put(v, cpu) for k, v in ins.items()}))
    got = kernel(**ins)
    rel = np.linalg.norm(got - exp) / np.linalg.norm(exp)
    print("Relative error:", rel)
